# revision 25
# baseline (speedup 1.0000x reference)
"""2D DWT (2-tap FFT reference) Trainium2 kernel.

The reference's FFT pipeline (pad to 256, circular conv, crop) reduces
algebraically to a 2x2 stencil per output:

    col pass:  v[r, c]   = wc1 * x[r, c] + wc0 * x[r+1, c]   (zero-ext r=224)
    row pass:  out[r, c] = wr1 * v[r, c] + wr0 * v[r, c+1]   (zero-ext c=224)

with (wc, wr) in {w_l, w_h}^2 for the four outputs: ll = (col l, row l),
lh = (col h, row l), hl = (col l, row h), hh = (col h, row h).

Kernel strategy (per core, 64 of the 512 independent images):
  * column pass on the tensor engine: v = S.T @ X with a banded stationary
    matrix S[p, m] = wc1*d(p==m) + wc0*d(p==m+1); image rows in SBUF
    partitions, two 112-row blocks per image; 16 images packed contiguously
    along the free dim (no pad columns), matmul windows of 2 images (448).
  * row pass fused into the PSUM drain: bounce v to persistent SBUF
    buffers that carry a zero pad column per image (stride 225), then
    out = beta*v + gamma*v_shift as ONE vector op per output over a
    [112, 2, 224] view - the shifted read lands on the zero pad at each
    image boundary, so no separate boundary handling. beta is +-1 for
    Haar-type filters (S is pre-scaled by wl1), giving plain add/sub;
    otherwise scalar_tensor_tensor / premultiply fallbacks are used.
  * all DRAM tensors use a custom per-core layout [.., half, p, g, c] so
    every DMA descriptor is one fully contiguous 14336-byte run per
    partition (the host pre/post-transposes in numpy); input DMAs go via
    the sync HWDGE ring (SWDGE input DMAs with >4KiB runs intermittently
    wedged the exec unit, so everything stays on HWDGE).
"""

import sys

for _p in ("/opt/trn_rl_repo", "/root/.axon_site/_ro/trn_rl_repo"):
    if _p not in sys.path:
        sys.path.append(_p)

import numpy as np

import concourse.bass as bass
import concourse.bacc as bacc
import concourse.mybir as mybir
from concourse import tile
from concourse.bass_utils import run_bass_kernel_spmd

N_CORES = 8
IMG = 64          # images per core  (512 total = 8 batch * 64 channels)
H = 224
W = 224
G = 8             # images per supertile
NSG = IMG // G    # supertile image-groups per core
FREE = G * W      # 1792 (contiguous, no pads)
SUBS = G // 2     # matmul windows per supertile (2 images each)
N = 2 * W         # 448 moving cols per window (<=512 fp32 limit)
M = 112           # output rows per matmul == half image height


def _row_coeffs(w_l, w_h):
    """Per-output (beta, gamma) for out = beta*v + gamma*v_sh, after the
    column matrices are scaled by alpha (so ll/lh get beta == 1)."""
    wl0, wl1 = float(w_l[0]), float(w_l[1])
    wh0, wh1 = float(w_h[0]), float(w_h[1])
    alpha = wl1 if abs(wl1) > 1e-30 else 1.0
    coeffs = [
        (wl1 / alpha, wl0 / alpha),   # ll: col l, row l
        (wl1 / alpha, wl0 / alpha),   # lh: col h, row l
        (wh1 / alpha, wh0 / alpha),   # hl: col l, row h
        (wh1 / alpha, wh0 / alpha),   # hh: col h, row h
    ]
    return alpha, coeffs


def _build_wmats(w_l, w_h):
    """Column-pass stationary matrices scaled by alpha, laid out
    [113, 4*112]: slot j = half*2 + f, f in {0: low, 1: high}."""
    alpha, _ = _row_coeffs(w_l, w_h)
    wm = np.zeros((113, 4 * M), np.float64)
    for half in range(2):
        K = 113 if half == 0 else 112
        for f, wc in enumerate([w_l, w_h]):
            S = np.zeros((113, M), np.float64)
            for m in range(M):
                S[m, m] = float(wc[1]) * alpha
                if m + 1 < K:
                    S[m + 1, m] = float(wc[0]) * alpha
            j = half * 2 + f
            wm[:, j * M:(j + 1) * M] = S
    return wm.astype(np.float32)


def _build_nc(beta_gamma) -> bass.Bass:
    """beta_gamma: list of 4 (beta, gamma) pairs baked as immediates."""
    nc = bacc.Bacc(
        "TRN2",
        target_bir_lowering=False,
        debug=False,
        num_devices=N_CORES,
    )
    f32 = mybir.dt.float32
    a = mybir.AluOpType
    # custom layouts: one contiguous (g, c) run per partition per DMA
    x = nc.dram_tensor("x", [NSG, 2, M, G, W], f32, kind="ExternalInput")
    wm = nc.dram_tensor("wm", [113, 4 * M], f32, kind="ExternalInput")
    out = nc.dram_tensor("out", [NSG, 2, M, 4, G, W], f32, kind="ExternalOutput")

    with tile.TileContext(nc) as tc:
        with (
            tc.tile_pool(name="wpool", bufs=1) as wpool,
            tc.tile_pool(name="xpool", bufs=8) as xpool,
            tc.tile_pool(name="opool", bufs=3) as opool,
            tc.tile_pool(name="tpool", bufs=3) as tpool,
            tc.tile_pool(name="pspool", bufs=3, space="PSUM") as pspool,
        ):
            wt = wpool.tile([113, 4 * M], f32)
            nc.sync.dma_start(out=wt[0:112, :], in_=wm[0:112, :])
            nc.sync.dma_start(out=wt[112:113, :], in_=wm[112:113, :])

            # persistent SBUF bounce buffers with a zero pad column per
            # image (stride 225) so the row-pass shift reads zero at the
            # image boundary; pads are zeroed once, manually rotated x3
            NBUF = 3
            svbufs = []
            for f in range(2):
                row = []
                for k in range(NBUF):
                    b = wpool.tile(
                        [M, 2 * (W + 1)], f32, tag=f"svb{f}_{k}",
                        name=f"svb{f}_{k}",
                    )
                    nc.vector.memset(
                        b[:, :].rearrange("p (i c) -> p i c", i=2)[:, :, W:W + 1],
                        0.0,
                    )
                    row.append(b)
                svbufs.append(row)

            for st in range(NSG * 2):
                sg, half = st // 2, st % 2
                K = 113 if half == 0 else 112

                xt = xpool.tile([113, FREE], f32, tag="xt", name=f"xt_{st}")
                nc.sync.dma_start(
                    out=xt[0:112, :],
                    in_=x[sg, half].rearrange("p g c -> p (g c)"),
                )
                if K == 113:
                    # row 112 of this block == row 0 of the next half-block
                    nc.sync.dma_start(
                        out=xt[112:113, :],
                        in_=x[sg, 1, 0:1].rearrange("p g c -> p (g c)"),
                    )

                otall = opool.tile(
                    [M, 4 * FREE], f32, tag="otall", name=f"otall_{st}"
                )
                for sub in range(SUBS):
                    s = sub * N
                    widx = st * SUBS + sub
                    svs = []
                    for f in range(2):  # 0: col-low, 1: col-high
                        v = pspool.tile(
                            [M, N], f32, tag=f"v{f}", name=f"v{f}_{st}_{sub}"
                        )
                        j0 = (half * 2 + f) * M
                        nc.tensor.matmul(
                            v[:, :],
                            wt[0:K, j0:j0 + M],
                            xt[0:K, s:s + N],
                            start=True,
                            stop=True,
                        )
                        # bounce to SBUF (one PSUM operand max per elemwise
                        # op); write only real cols, pads stay zero
                        sv = svbufs[f][widx % NBUF]
                        sv2 = sv[:, :].rearrange("p (i c) -> p i c", i=2)
                        nc.scalar.copy(
                            sv2[:, :, 0:W],
                            v[:, :].rearrange("p (i c) -> p i c", i=2),
                        )
                        svs.append(sv2)
                    # row pass over full 224-col views; the shifted read
                    # hits the zero pad at each image boundary
                    for o, sv2 in ((0, svs[0]), (1, svs[1]), (2, svs[0]), (3, svs[1])):
                        beta, gamma = beta_gamma[o]
                        ob = o * FREE + s
                        ot2 = otall[:, ob:ob + N].rearrange("p (i c) -> p i c", i=2)
                        sh = sv2[:, :, 1:W + 1]
                        base = sv2[:, :, 0:W]
                        plain = abs(gamma - 1.0) < 1e-12  # out = v_sh +- v
                        # keep Pool engine SWDGE-only (mixing Q7 compute with
                        # SWDGE descriptor generation hung intermittently)
                        eng = nc.vector
                        if plain and abs(beta - 1.0) < 1e-12:
                            eng.tensor_add(ot2[:, :, :], sh, base)
                        elif plain and abs(beta + 1.0) < 1e-12:
                            eng.tensor_sub(ot2[:, :, :], sh, base)
                        elif abs(beta - 1.0) < 1e-12:
                            eng.scalar_tensor_tensor(
                                ot2[:, :, :], sh, float(gamma), base,
                                a.mult, a.add,
                            )
                        elif abs(beta + 1.0) < 1e-12:
                            eng.scalar_tensor_tensor(
                                ot2[:, :, :], sh, float(gamma), base,
                                a.mult, a.subtract,
                            )
                        else:
                            tmp = tpool.tile(
                                [M, N], f32, tag="tmp", name=f"tmp{o}_{st}_{sub}"
                            )
                            tmp2 = tmp[:, :].rearrange("p (i c) -> p i c", i=2)
                            eng.tensor_scalar_mul(tmp2[:, :, :], base, float(beta))
                            eng.scalar_tensor_tensor(
                                ot2[:, :, :], sh, float(gamma), tmp2[:, :, :],
                                a.mult, a.add,
                            )

                nc.sync.dma_start(
                    out=out[sg, half].rearrange("p o g c -> p (o g c)"),
                    in_=otall[:, :],
                )
    nc.compile()
    return nc


_NC_CACHE: dict = {}


def _get_nc(w_l, w_h) -> bass.Bass:
    _, coeffs = _row_coeffs(w_l, w_h)
    key = tuple(coeffs[o] for o in range(4))
    if key not in _NC_CACHE:
        _NC_CACHE[key] = _build_nc(coeffs)
    return _NC_CACHE[key]


def kernel(x, w_l, w_h, **run_kwargs):
    x = np.asarray(x, dtype=np.float32)
    w_l = np.asarray(w_l, dtype=np.float32).reshape(-1)
    w_h = np.asarray(w_h, dtype=np.float32).reshape(-1)
    assert x.shape == (8, 64, H, W), x.shape
    assert w_l.shape == (2,) and w_h.shape == (2,)

    wm = _build_wmats(w_l, w_h)
    # per-core relayout: (IMG, 224, 224) -> (NSG, half, p, g, c)
    xs = x.reshape(N_CORES, NSG, G, 2, M, W).transpose(0, 1, 3, 4, 2, 5)
    in_maps = [
        {"x": np.ascontiguousarray(xs[i]), "wm": wm} for i in range(N_CORES)
    ]
    res = run_bass_kernel_spmd(
        _get_nc(w_l, w_h), in_maps, core_ids=list(range(N_CORES)), **run_kwargs
    )
    # gather + inverse relayout: [NSG, half, p, o, g, c] -> [4, IMG, H, W]
    full = np.stack([r["out"] for r in res.results], axis=0)
    # full: [core, NSG, 2, M, 4, G, W] -> [o, core, NSG, G, 2, M, W]
    full = full.transpose(4, 0, 1, 5, 2, 3, 6).reshape(4, 8, 64, H, W)
    if run_kwargs:
        kernel.last_result = res  # expose profile info to test harnesses
    return (full[0], full[1], full[2], full[3])


# revision 28
# speedup vs baseline: 1.0395x; 1.0395x over previous
"""2D DWT (2-tap FFT reference) Trainium2 kernel.

The reference's FFT pipeline (pad to 256, circular conv, crop) reduces
algebraically to a 2x2 stencil per output:

    col pass:  v[r, c]   = wc1 * x[r, c] + wc0 * x[r+1, c]   (zero-ext r=224)
    row pass:  out[r, c] = wr1 * v[r, c] + wr0 * v[r, c+1]   (zero-ext c=224)

with (wc, wr) in {w_l, w_h}^2 for the four outputs: ll = (col l, row l),
lh = (col h, row l), hl = (col l, row h), hh = (col h, row h).

Kernel strategy (per core, 64 of the 512 independent images):
  * column pass on the tensor engine: v = S.T @ X with a banded stationary
    matrix S[p, m] = wc1*d(p==m) + wc0*d(p==m+1); image rows in SBUF
    partitions, two 112-row blocks per image; 16 images packed contiguously
    along the free dim (no pad columns), matmul windows of 2 images (448).
  * row pass fused into the PSUM drain: bounce v to persistent SBUF
    buffers that carry a zero pad column per image (stride 225), then
    out = beta*v + gamma*v_shift as ONE vector op per output over a
    [112, 2, 224] view - the shifted read lands on the zero pad at each
    image boundary, so no separate boundary handling. beta is +-1 for
    Haar-type filters (S is pre-scaled by wl1), giving plain add/sub;
    otherwise scalar_tensor_tensor / premultiply fallbacks are used.
  * all DRAM tensors use a custom per-core layout [.., half, p, g, c] so
    every DMA descriptor is one fully contiguous 14336-byte run per
    partition (the host pre/post-transposes in numpy); input DMAs go via
    the sync HWDGE ring (SWDGE input DMAs with >4KiB runs intermittently
    wedged the exec unit, so everything stays on HWDGE).
"""

import sys

for _p in ("/opt/trn_rl_repo", "/root/.axon_site/_ro/trn_rl_repo"):
    if _p not in sys.path:
        sys.path.append(_p)

import numpy as np

import concourse.bass as bass
import concourse.bacc as bacc
import concourse.mybir as mybir
from concourse import tile
from concourse.bass_utils import run_bass_kernel_spmd

N_CORES = 8
IMG = 64          # images per core  (512 total = 8 batch * 64 channels)
H = 224
W = 224
G = 8             # images per supertile
NSG = IMG // G    # supertile image-groups per core
FREE = G * W      # 1792 (contiguous, no pads)
SUBS = G // 2     # matmul windows per supertile (2 images each)
N = 2 * W         # 448 moving cols per window (<=512 fp32 limit)
M = 112           # output rows per matmul == half image height


def _row_coeffs(w_l, w_h):
    """Per-output (beta, gamma) for out = beta*v + gamma*v_sh, after the
    column matrices are scaled by alpha (so ll/lh get beta == 1)."""
    wl0, wl1 = float(w_l[0]), float(w_l[1])
    wh0, wh1 = float(w_h[0]), float(w_h[1])
    alpha = wl1 if abs(wl1) > 1e-30 else 1.0
    coeffs = [
        (wl1 / alpha, wl0 / alpha),   # ll: col l, row l
        (wl1 / alpha, wl0 / alpha),   # lh: col h, row l
        (wh1 / alpha, wh0 / alpha),   # hl: col l, row h
        (wh1 / alpha, wh0 / alpha),   # hh: col h, row h
    ]
    return alpha, coeffs


def _build_wmats(w_l, w_h):
    """Column-pass stationary matrices scaled by alpha, laid out
    [113, 4*112]: slot j = half*2 + f, f in {0: low, 1: high}."""
    alpha, _ = _row_coeffs(w_l, w_h)
    wm = np.zeros((113, 4 * M), np.float64)
    for half in range(2):
        K = 113 if half == 0 else 112
        for f, wc in enumerate([w_l, w_h]):
            S = np.zeros((113, M), np.float64)
            for m in range(M):
                S[m, m] = float(wc[1]) * alpha
                if m + 1 < K:
                    S[m + 1, m] = float(wc[0]) * alpha
            j = half * 2 + f
            wm[:, j * M:(j + 1) * M] = S
    return wm.astype(np.float32)


def _build_nc(beta_gamma) -> bass.Bass:
    """beta_gamma: list of 4 (beta, gamma) pairs baked as immediates."""
    nc = bacc.Bacc(
        "TRN2",
        target_bir_lowering=False,
        debug=False,
        num_devices=N_CORES,
    )
    f32 = mybir.dt.float32
    a = mybir.AluOpType
    # custom layouts: one contiguous (g, c) run per partition per DMA
    x = nc.dram_tensor("x", [NSG, 2, M, G, W], f32, kind="ExternalInput")
    wm = nc.dram_tensor("wm", [113, 4 * M], f32, kind="ExternalInput")
    out = nc.dram_tensor("out", [NSG, 2, M, 4, G, W], f32, kind="ExternalOutput")

    with tile.TileContext(nc) as tc:
        with (
            tc.tile_pool(name="wpool", bufs=1) as wpool,
            tc.tile_pool(name="xpool", bufs=4) as xpool,
            tc.tile_pool(name="opool", bufs=3) as opool,
            tc.tile_pool(name="tpool", bufs=3) as tpool,
            tc.tile_pool(name="pspool", bufs=3, space="PSUM") as pspool,
        ):
            wt = wpool.tile([113, 4 * M], f32)
            nc.sync.dma_start(out=wt[0:112, :], in_=wm[0:112, :])
            nc.sync.dma_start(out=wt[112:113, :], in_=wm[112:113, :])

            # persistent SBUF bounce buffers with a zero pad column per
            # image (stride 225) so the row-pass shift reads zero at the
            # image boundary; pads are zeroed once, manually rotated x3
            NBUF = 3
            svbufs = []
            for f in range(2):
                row = []
                for k in range(NBUF):
                    b = wpool.tile(
                        [M, 2 * (W + 1)], f32, tag=f"svb{f}_{k}",
                        name=f"svb{f}_{k}",
                    )
                    nc.vector.memset(
                        b[:, :].rearrange("p (i c) -> p i c", i=2)[:, :, W:W + 1],
                        0.0,
                    )
                    row.append(b)
                svbufs.append(row)

            for st in range(NSG * 2):
                sg, half = st // 2, st % 2
                K = 113 if half == 0 else 112

                xt = xpool.tile([113, FREE], f32, tag="xt", name=f"xt_{st}")
                nc.sync.dma_start(
                    out=xt[0:112, :],
                    in_=x[sg, half].rearrange("p g c -> p (g c)"),
                )
                if K == 113:
                    # row 112 of this block == row 0 of the next half-block
                    nc.sync.dma_start(
                        out=xt[112:113, :],
                        in_=x[sg, 1, 0:1].rearrange("p g c -> p (g c)"),
                    )

                otall = opool.tile(
                    [M, 4 * FREE], f32, tag="otall", name=f"otall_{st}"
                )
                for sub in range(SUBS):
                    s = sub * N
                    widx = st * SUBS + sub
                    svs = []
                    for f in range(2):  # 0: col-low, 1: col-high
                        v = pspool.tile(
                            [M, N], f32, tag=f"v{f}", name=f"v{f}_{st}_{sub}"
                        )
                        j0 = (half * 2 + f) * M
                        nc.tensor.matmul(
                            v[:, :],
                            wt[0:K, j0:j0 + M],
                            xt[0:K, s:s + N],
                            start=True,
                            stop=True,
                        )
                        # bounce to SBUF (one PSUM operand max per elemwise
                        # op); write only real cols, pads stay zero
                        sv = svbufs[f][widx % NBUF]
                        sv2 = sv[:, :].rearrange("p (i c) -> p i c", i=2)
                        nc.scalar.copy(
                            sv2[:, :, 0:W],
                            v[:, :].rearrange("p (i c) -> p i c", i=2),
                        )
                        svs.append(sv2)
                    # row pass over full 224-col views; the shifted read
                    # hits the zero pad at each image boundary
                    for o, sv2 in ((0, svs[0]), (1, svs[1]), (2, svs[0]), (3, svs[1])):
                        beta, gamma = beta_gamma[o]
                        ob = o * FREE + s
                        ot2 = otall[:, ob:ob + N].rearrange("p (i c) -> p i c", i=2)
                        sh = sv2[:, :, 1:W + 1]
                        base = sv2[:, :, 0:W]
                        plain = abs(gamma - 1.0) < 1e-12  # out = v_sh +- v
                        # keep Pool engine SWDGE-only (mixing Q7 compute with
                        # SWDGE descriptor generation hung intermittently)
                        eng = nc.vector
                        if plain and abs(beta - 1.0) < 1e-12:
                            eng.tensor_add(ot2[:, :, :], sh, base)
                        elif plain and abs(beta + 1.0) < 1e-12:
                            eng.tensor_sub(ot2[:, :, :], sh, base)
                        elif abs(beta - 1.0) < 1e-12:
                            eng.scalar_tensor_tensor(
                                ot2[:, :, :], sh, float(gamma), base,
                                a.mult, a.add,
                            )
                        elif abs(beta + 1.0) < 1e-12:
                            eng.scalar_tensor_tensor(
                                ot2[:, :, :], sh, float(gamma), base,
                                a.mult, a.subtract,
                            )
                        else:
                            tmp = tpool.tile(
                                [M, N], f32, tag="tmp", name=f"tmp{o}_{st}_{sub}"
                            )
                            tmp2 = tmp[:, :].rearrange("p (i c) -> p i c", i=2)
                            eng.tensor_scalar_mul(tmp2[:, :, :], base, float(beta))
                            eng.scalar_tensor_tensor(
                                ot2[:, :, :], sh, float(gamma), tmp2[:, :, :],
                                a.mult, a.add,
                            )

                nc.sync.dma_start(
                    out=out[sg, half].rearrange("p o g c -> p (o g c)"),
                    in_=otall[:, :],
                )
    nc.compile()
    return nc


_NC_CACHE: dict = {}


def _get_nc(w_l, w_h) -> bass.Bass:
    _, coeffs = _row_coeffs(w_l, w_h)
    key = tuple(coeffs[o] for o in range(4))
    if key not in _NC_CACHE:
        _NC_CACHE[key] = _build_nc(coeffs)
    return _NC_CACHE[key]


def kernel(x, w_l, w_h, **run_kwargs):
    x = np.asarray(x, dtype=np.float32)
    w_l = np.asarray(w_l, dtype=np.float32).reshape(-1)
    w_h = np.asarray(w_h, dtype=np.float32).reshape(-1)
    assert x.shape == (8, 64, H, W), x.shape
    assert w_l.shape == (2,) and w_h.shape == (2,)

    wm = _build_wmats(w_l, w_h)
    # per-core relayout: (IMG, 224, 224) -> (NSG, half, p, g, c)
    xs = x.reshape(N_CORES, NSG, G, 2, M, W).transpose(0, 1, 3, 4, 2, 5)
    in_maps = [
        {"x": np.ascontiguousarray(xs[i]), "wm": wm} for i in range(N_CORES)
    ]
    res = run_bass_kernel_spmd(
        _get_nc(w_l, w_h), in_maps, core_ids=list(range(N_CORES)), **run_kwargs
    )
    # gather + inverse relayout: [NSG, half, p, o, g, c] -> [4, IMG, H, W]
    full = np.stack([r["out"] for r in res.results], axis=0)
    # full: [core, NSG, 2, M, 4, G, W] -> [o, core, NSG, G, 2, M, W]
    full = full.transpose(4, 0, 1, 5, 2, 3, 6).reshape(4, 8, 64, H, W)
    if run_kwargs:
        kernel.last_result = res  # expose profile info to test harnesses
    return (full[0], full[1], full[2], full[3])
